# revision 27
# baseline (speedup 1.0000x reference)
"""Trainium2 Bass kernel for DepthwiseSeparableConv (depthwise 3x3 shared-kernel
conv -> channels-last memory-reinterpret -> pointwise 1x1 conv -> ReLU -> sync
BatchNorm), data-parallel over batch across 8 NeuronCores.

Self-contained: hardcodes shapes from the problem spec; imports only the
system-installed `concourse` (Bass/Tile) stack.

Key layout trick vs the naive scheme: conv blocks are STRIDED, block l holds
spatial rows n = 28q + l (q = 0..111 partitions, l = 0..27 blocks). The 3x3
stencil still reduces to 3 banded matmuls per block (bands at q-offsets
{-2,0,+2} <-> dh, block delta <-> dw); h-borders drop out automatically via
band clipping, w-borders only touch the two wrap matrices (parity masks on q).
Because q is the SLOW index of the channels-last flat order
(F = 3584 q + 128 l + c), the conv output can be written to DRAM with 3584B
contiguous runs (full DMA speed) and read back as y = [128, 3136] contiguous,
realizing the reference's memory reinterpretation with ~1/3 the DMA cost of a
256B-chunked bounce.

Per core (8 of 64 batches, 2 groups of 4):
  1. Load x[b] [128c, 3136n] bf16. PE-transpose strided column sets
     x[:, l::28] -> xt_l [112q, 4b*128c] bf16 (DVE copies PSUM->SBUF).
  2. Conv: ps_l = A_{-1}^T xt_{l-1} + A_0^T xt_l + A_{+1}^T xt_{l+1} (wrap
     matrices for l=0/27), ACT copies f32 PSUM -> bf16 zg half-tiles.
  3. Bounce zg -> DRAM (4 half-writes/group, 3584B runs) and read back
     y[b] = [128, 3136] bf16 (contiguous).
  4. Pointwise out = pw @ y on PE (bf16 in, f32 accum); ReLU fused into
     PSUM->SBUF with per-channel sum accumulators; squares via second pass
     with sumsq accumulators. Pre-BN activations stay resident in SBUF bf16.
  5. Per-channel (sum, sumsq) AllGather across 8 cores + local reduce (exact
     sync-BN), affine applied on DVE/ACT/Pool, bf16 written out (host widens
     to f32).
"""

import os
import numpy as np
from contextlib import ExitStack

import concourse.bass as bass
import concourse.bacc as bacc
import concourse.mybir as mybir
from concourse import tile
from concourse.bass_utils import run_bass_kernel_spmd

F32 = mybir.dt.float32
BF16 = mybir.dt.bfloat16

B, CIN, COUT, H, W = 64, 128, 256, 56, 56
HW = H * W              # 3136
Q = 112                 # rows per strided block (n = 28q + l)
NBLK = 28               # blocks per image
HBLK = NBLK // 2        # 14 blocks per z half-tile
NCORES = 8
BPC = B // NCORES       # 8 batches per core
GB = 4                  # batches per conv group
NGRP = BPC // GB        # 2
NCH = 448               # pointwise moving chunk
EPS = 1e-5
NTOT = float(B * HW)    # BN count
NSL = 4                 # stats slots per (batch, oc)


def _build_A(nc, k_sb, const, tmp_pool):
    """Build the 5 banded conv matrices [128part(q_in, 112 used), 112(q_out)]
    bf16: A[dw] plain (bands q_in-q_out = 2*dh, coeff k[3*(dh+1)+(dw+1)]),
    plus wrap variants A[-1]w (shifts 2dh-1, odd q_out only; used by block 0
    reading xt_27) and A[+1]w (shifts 2dh+1, even q_out only; block 27
    reading xt_0)."""
    specs = {}
    # block-0's matrices first so conv can start as soon as possible
    specs["Am1w"] = [(2 * dh - 1, 3 * (dh + 1) + 0, 1) for dh in (-1, 0, 1)]
    for dw in (0, 1, -1):
        specs[f"A{dw}"] = [(2 * dh, 3 * (dh + 1) + (dw + 1), None)
                           for dh in (-1, 0, 1)]
    specs["Ap1w"] = [(2 * dh + 1, 3 * (dh + 1) + 2, 0) for dh in (-1, 0, 1)]

    out = {}
    for nm, taps in specs.items():
        parts = []
        for shift, k9i, parity in taps:
            t = tmp_pool.tile([128, Q], F32, tag="abuild", bufs=16,
                              name=f"ab_{nm}_{shift}")
            # diagonal: keep k where p - f - shift == 0 (f = 2*i1 + i2)
            nc.gpsimd.affine_select(
                out=t[:], in_=k_sb[:, k9i:k9i + 1].broadcast_to((128, Q)),
                pattern=[[-2, 56], [-1, 2]], base=-shift,
                compare_op=mybir.AluOpType.is_equal, fill=0.0,
                channel_multiplier=1)
            if parity is not None:
                # keep only columns with f % 2 == parity (value = i2 = f % 2)
                nc.gpsimd.affine_select(
                    out=t[:], in_=t[:], pattern=[[0, 56], [1, 2]],
                    base=-parity, compare_op=mybir.AluOpType.is_equal,
                    fill=0.0, channel_multiplier=0)
            parts.append(t)
        af = tmp_pool.tile([128, Q], F32, tag="abuildacc", bufs=2,
                           name=f"af_{nm}")
        nc.vector.tensor_tensor(af[:], parts[0][:], parts[1][:],
                                mybir.AluOpType.add)
        nc.vector.tensor_tensor(af[:], af[:], parts[2][:],
                                mybir.AluOpType.add)
        ar = const.tile([128, Q], BF16, tag=f"Ar_{nm}", name=f"Ar_{nm}")
        nc.vector.tensor_copy(ar[:], af[:])
        out[nm] = ar
    return out


def build_nc():
    nc = bacc.Bacc(num_devices=NCORES)

    x_in = nc.declare_dram_parameter("x", [BPC, CIN, HW], BF16, isOutput=False)
    dwk = nc.declare_dram_parameter("dwk", [9], F32, isOutput=False)
    pwt = nc.declare_dram_parameter("pwT", [CIN, COUT], F32, isOutput=False)
    gam = nc.declare_dram_parameter("gamma", [COUT], F32, isOutput=False)
    bet = nc.declare_dram_parameter("beta", [COUT], F32, isOutput=False)
    out = nc.declare_dram_parameter("out", [BPC, COUT, HW], BF16, isOutput=True)

    with ExitStack() as ctx:
        tc = ctx.enter_context(tile.TileContext(nc))
        const = ctx.enter_context(tc.tile_pool(name="const", bufs=1))
        xtpool = ctx.enter_context(tc.tile_pool(name="xt", bufs=4))
        zgpool = ctx.enter_context(tc.tile_pool(name="zg", bufs=2))
        ypool = ctx.enter_context(tc.tile_pool(name="y", bufs=3))
        respool = ctx.enter_context(tc.tile_pool(name="res", bufs=2 * BPC))
        scpool = ctx.enter_context(tc.tile_pool(name="scr", bufs=1))
        ps_t = ctx.enter_context(tc.tile_pool(name="ps_t", bufs=2, space="PSUM"))
        ps_m = ctx.enter_context(tc.tile_pool(name="ps_m", bufs=3, space="PSUM"))
        dram = ctx.enter_context(tc.tile_pool(name="dram", bufs=1, space="DRAM"))

        no_cc = bool(os.environ.get("BASS_NO_CC"))

        # ---- constants ----
        k_sb = const.tile([128, 9], F32, tag="k")
        nc.sync.dma_start(k_sb[:], dwk.ap().partition_broadcast(128))

        ident = const.tile([128, 128], BF16, tag="ident")
        pw_sb = const.tile([128, COUT], BF16, tag="pw")
        gb_sb = const.tile([128, 4], F32, tag="gb")
        nc.sync.dma_start(gb_sb[:, 0:2], gam.ap().rearrange("(j p) -> p j", p=128))
        nc.sync.dma_start(gb_sb[:, 2:4], bet.ap().rearrange("(j p) -> p j", p=128))

        ones = const.tile([128, 1], F32, tag="ones")
        nc.gpsimd.memset(ones[:], 1.0)
        with tc.tile_pool(name="abuild", bufs=2) as tmp_pool:
            pw_f32 = tmp_pool.tile([128, COUT], F32, tag="pwf", bufs=1)
            nc.sync.dma_start(pw_f32[:], pwt[:, :])
            nc.vector.tensor_copy(pw_sb[:], pw_f32[:])
            identf = tmp_pool.tile([128, 128], F32, tag="identf", bufs=1)
            nc.gpsimd.affine_select(
                out=identf[:], in_=ones[:].broadcast_to((128, 128)),
                pattern=[[-1, 128]], base=0,
                compare_op=mybir.AluOpType.is_equal, fill=0.0,
                channel_multiplier=1)
            nc.gpsimd.tensor_copy(ident[:], identf[:])
            A = _build_A(nc, k_sb, const, tmp_pool)

        xpool = ctx.enter_context(tc.tile_pool(name="x", bufs=6))

        # stats slots: sums one column per (batch, chunk-group); sumsq one
        # column per (batch, oc) unit (single full-width Square per unit)
        sums = [const.tile([128, BPC * NSL], F32, tag=f"sum{oc}",
                           name=f"sums{oc}") for oc in range(2)]
        sqs = [const.tile([128, BPC], F32, tag=f"sq{oc}",
                          name=f"sqs{oc}") for oc in range(2)]

        # DRAM bounce scratch: zscr[g][b4] flat == y[b] flat (n-major, 128c)
        zscr = [dram.tile([GB, Q, NBLK * 128], BF16, tag=f"zg{g}",
                          name=f"zscr{g}") for g in range(NGRP)]
        # two-phase stats all-gather: one per conv group so the first hides
        # under the second group's pointwise work
        st_in = [dram.tile([128, 4], F32, tag=f"stin{g}", name=f"st_in{g}")
                 for g in range(NGRP)]
        st_gather = [dram.tile([NCORES, 128, 4], F32, tag=f"stg{g}",
                               name=f"st_g{g}") for g in range(NGRP)]

        res_tiles = [[None] * 2 for _ in range(BPC)]
        x_tiles = [None] * BPC
        y_tiles = [None] * BPC

        def load_x(b):
            xt_ = xpool.tile([128, HW], BF16, tag="x", name=f"xh{b}")
            nc.sync.dma_start(xt_[:], x_in[b, :, :])
            x_tiles[b] = xt_

        for b in range(GB + 1):
            load_x(b)

        # ---------- conv group emission (generator: yields per block) ----------
        def emit_conv(g):
            xt_tiles = {}
            zgh = [None, None]
            b0 = GB * g

            def transpose_block(l):
                tps = ps_t.tile([Q, GB * 128], BF16, tag="tp")
                for b4 in range(GB):
                    xv = (x_tiles[b0 + b4][:, :]
                          .rearrange("c (q l) -> l c q", q=Q, l=NBLK)[l])
                    nc.tensor.transpose(
                        tps[:, 128 * b4:128 * (b4 + 1)], xv, ident[:])
                xt_sb = xtpool.tile([Q, GB * 128], BF16,
                                    tag=("xt27" if l == 27 else
                                         "xt0" if l == 0 else "xt"),
                                    bufs=(1 if l in (0, 27) else 4),
                                    name=f"xt{g}_{l}")
                nc.vector.tensor_copy(xt_sb[:], tps[:])
                xt_tiles[l] = xt_sb

            def conv_block(l):
                h = l // HBLK
                if zgh[h] is None:
                    zgh[h] = zgpool.tile([Q, GB * HBLK * 128], BF16, tag="zg",
                                         name=f"zgt{g}_{h}")
                psm = ps_m.tile([128, 1024], F32, tag="m", name="psc")
                ps = psm[0:Q, 0:GB * 128]
                if l == 0:
                    mats = [(A["Am1w"], 27), (A["A0"], 0), (A["A1"], 1)]
                elif l == NBLK - 1:
                    mats = [(A["A-1"], l - 1), (A["A0"], l), (A["Ap1w"], 0)]
                else:
                    mats = [(A["A-1"], l - 1), (A["A0"], l), (A["A1"], l + 1)]
                for idx, (a, src) in enumerate(mats):
                    nc.tensor.matmul(
                        ps[:], a[0:Q, :], xt_tiles[src][:],
                        start=(idx == 0), stop=(idx == 2))
                zv = (zgh[h][:, :]
                      .rearrange("q (b l c) -> l q b c", b=GB, l=HBLK, c=128)
                      [l % HBLK])
                nc.scalar.activation(zv, ps[:],
                                     mybir.ActivationFunctionType.Copy)
                if l % HBLK == HBLK - 1:
                    for bp in range(2):
                        nc.sync.dma_start(
                            zscr[g][2 * bp:2 * bp + 2,
                                    :, 1792 * h:1792 * (h + 1)]
                            .rearrange("b q s -> q b s"),
                            zgh[h][:, :].rearrange("q (b lc) -> q b lc", b=GB)
                            [:, 2 * bp:2 * bp + 2, :])
                    zgh[h] = None

            transpose_block(27)
            yield
            transpose_block(0)
            yield
            for l in range(1, NBLK):
                transpose_block(l)
                conv_block(l - 1)
                yield
            conv_block(NBLK - 1)
            yield

        # ---------- pointwise emission (per (b, oc) unit) ----------
        def emit_pointwise(g):
            for b4 in range(GB):
                b = GB * g + b4
                y_sb = ypool.tile([128, HW], BF16, tag="y", name=f"y{b}")
                nc.sync.dma_start(
                    y_sb[:],
                    zscr[g][b4].flatten().rearrange("(p n) -> p n", p=128))
                y_tiles[b] = y_sb
                for oc in range(2):
                    res = respool.tile([128, HW], BF16, tag="res")
                    res_tiles[b][oc] = res
                    for jj, js in enumerate([(0, 1), (2, 3), (4, 5), (6,)]):
                        w = NCH * len(js)
                        ps = ps_m.tile([128, 1024], F32, tag="m", name="psp")
                        for k, j in enumerate(js):
                            nc.tensor.matmul(
                                ps[:, 512 * k:512 * k + NCH],
                                pw_sb[:, 128 * oc:128 * (oc + 1)],
                                y_sb[:, NCH * j:NCH * (j + 1)],
                                start=True, stop=True)
                        slot = b * NSL + jj
                        j0 = js[0]
                        if len(js) == 2:
                            ps_in = (ps[:].rearrange("p (k c) -> p k c", c=512)
                                     [:, :, 0:NCH])
                            rs = (res[:, NCH * j0:NCH * j0 + w]
                                  .rearrange("p (k c) -> p k c", c=NCH))
                        else:
                            ps_in = ps[:, 0:NCH]
                            rs = res[:, NCH * j0:NCH * j0 + w]
                        nc.vector.tensor_scalar(
                            rs, ps_in, 0.0, 0.0,
                            mybir.AluOpType.max, mybir.AluOpType.add,
                            accum_out=sums[oc][:, slot:slot + 1])
                    # one full-width square pass on ACT -> sumsq
                    sc = scpool.tile([128, HW], BF16, tag="scr")
                    nc.scalar.activation(
                        sc[:], res[:],
                        mybir.ActivationFunctionType.Square,
                        accum_out=sqs[oc][:, b:b + 1])
                    yield

        # per-group stats launch: reduce this group's slots and AllGather
        red = [const.tile([128, 4], F32, tag=f"red{g}", name=f"red{g}")
               for g in range(NGRP)]
        ag = [const.tile([128, 4 * NCORES], F32, tag=f"ag{g}", name=f"agt{g}")
              for g in range(NGRP)]

        def emit_stats_gather(g):
            s0, s1 = GB * NSL * g, GB * NSL * (g + 1)
            for oc in range(2):
                nc.vector.tensor_reduce(red[g][:, oc:oc + 1],
                                        sums[oc][:, s0:s1],
                                        axis=mybir.AxisListType.X,
                                        op=mybir.AluOpType.add)
                nc.vector.tensor_reduce(red[g][:, 2 + oc:3 + oc],
                                        sqs[oc][:, GB * g:GB * (g + 1)],
                                        axis=mybir.AxisListType.X,
                                        op=mybir.AluOpType.add)
            nc.gpsimd.dma_start(st_in[g][:], red[g][:])
            if no_cc:
                for r in range(NCORES):
                    nc.gpsimd.dma_start(st_gather[g][r], st_in[g][:])
            else:
                nc.gpsimd.collective_compute(
                    "AllGather", mybir.AluOpType.bypass,
                    replica_groups=[list(range(NCORES))],
                    ins=[st_in[g][:].opt()], outs=[st_gather[g][:].opt()])
            nc.gpsimd.dma_start(
                ag[g][:], st_gather[g][:].rearrange("r p f -> p r f"))

        # ---------- schedule: g0 conv | g1 conv x g0 pw interleave | g1 pw ----
        for _ in emit_conv(0):
            pass
        for b in range(GB + 1, BPC):
            load_x(b)
        pw0 = emit_pointwise(0)
        step = 0
        for _ in emit_conv(1):
            step += 1
            if step % 5 == 0:
                next(pw0, None)
        # remaining pw0 units drain into the g1-bounce seam
        for _ in pw0:
            pass
        emit_stats_gather(0)
        for _ in emit_pointwise(1):
            pass
        emit_stats_gather(1)

        # ---- combine gathered stats -> affine params ----
        me = const.tile([128, 4], F32, tag="me")    # mean0 mean1 msq0 msq1
        var = const.tile([128, 2], F32, tag="var")
        std = const.tile([128, 2], F32, tag="std")
        rstd = const.tile([128, 2], F32, tag="rstd")
        sc_b = const.tile([128, 4], F32, tag="scb")  # scale0 scale1 nbias0 nbias1

        a0, a1 = ag[0], ag[1]
        nc.vector.tensor_tensor(a0[:, 0:32], a0[:, 0:32], a1[:, 0:32],
                                mybir.AluOpType.add)
        nc.vector.tensor_tensor(a0[:, 0:16], a0[:, 0:16], a0[:, 16:32],
                                mybir.AluOpType.add)
        nc.vector.tensor_tensor(a0[:, 0:8], a0[:, 0:8], a0[:, 8:16],
                                mybir.AluOpType.add)
        nc.vector.tensor_tensor(a0[:, 0:4], a0[:, 0:4], a0[:, 4:8],
                                mybir.AluOpType.add)

        nc.vector.tensor_scalar(me[:], a0[:, 0:4], 1.0 / NTOT, None,
                                mybir.AluOpType.mult)
        nc.vector.tensor_tensor(var[:], me[:, 0:2], me[:, 0:2],
                                mybir.AluOpType.mult)
        nc.vector.tensor_tensor(var[:], me[:, 2:4], var[:],
                                mybir.AluOpType.subtract)
        nc.vector.tensor_scalar(var[:], var[:], EPS, None,
                                mybir.AluOpType.add)
        nc.scalar.activation(std[:], var[:],
                             mybir.ActivationFunctionType.Sqrt)
        nc.vector.reciprocal(rstd[:], std[:])
        nc.vector.tensor_tensor(sc_b[:, 0:2], rstd[:], gb_sb[:, 0:2],
                                mybir.AluOpType.mult)
        nc.vector.tensor_tensor(sc_b[:, 2:4], me[:, 0:2], sc_b[:, 0:2],
                                mybir.AluOpType.mult)
        nc.vector.tensor_tensor(sc_b[:, 2:4], gb_sb[:, 2:4], sc_b[:, 2:4],
                                mybir.AluOpType.subtract)

        # ---- phase 2: in-place affine on res + bf16 writeout ----
        for b in range(BPC):
            for oc in range(2):
                r = res_tiles[b][oc]
                idx = 2 * b + oc
                if idx % 8 == 3:
                    nc.gpsimd.tensor_scalar(
                        r[:], r[:],
                        sc_b[:, oc:oc + 1], sc_b[:, 2 + oc:3 + oc],
                        mybir.AluOpType.mult, mybir.AluOpType.add)
                elif idx % 4 == 1:
                    nc.scalar.activation(
                        r[:], r[:],
                        mybir.ActivationFunctionType.Identity,
                        bias=sc_b[:, 2 + oc:3 + oc],
                        scale=sc_b[:, oc:oc + 1])
                else:
                    nc.vector.tensor_scalar(
                        r[:], r[:],
                        sc_b[:, oc:oc + 1], sc_b[:, 2 + oc:3 + oc],
                        mybir.AluOpType.mult, mybir.AluOpType.add)
                nc.sync.dma_start(out[b, 128 * oc:128 * (oc + 1), :], r[:])

    nc.finalize()
    return nc


_NC_CACHE = []


def kernel(x, dw_w, pw_w, gamma, beta):
    import ml_dtypes
    x = np.ascontiguousarray(
        np.asarray(x, dtype=np.float32).astype(ml_dtypes.bfloat16)
    ).reshape(B, CIN, HW)
    dwk = np.ascontiguousarray(np.asarray(dw_w, dtype=np.float32)).reshape(9)
    pwT = np.ascontiguousarray(np.asarray(pw_w, dtype=np.float32).T)
    gamma = np.ascontiguousarray(np.asarray(gamma, dtype=np.float32))
    beta = np.ascontiguousarray(np.asarray(beta, dtype=np.float32))

    if not _NC_CACHE:
        _NC_CACHE.append(build_nc())
    nc = _NC_CACHE[0]

    in_maps = []
    for r in range(NCORES):
        shard = np.ascontiguousarray(x[r * BPC:(r + 1) * BPC])
        in_maps.append({"x": shard, "dwk": dwk, "pwT": pwT,
                        "gamma": gamma, "beta": beta})

    br = run_bass_kernel_spmd(nc, in_maps, list(range(NCORES)))
    outs = [np.asarray(br.results[r]["out"], dtype=np.float32)
            .reshape(BPC, COUT, H, W) for r in range(NCORES)]
    return np.concatenate(outs, axis=0)


# revision 28
# speedup vs baseline: 1.0290x; 1.0290x over previous
"""Trainium2 Bass kernel for DepthwiseSeparableConv (depthwise 3x3 shared-kernel
conv -> channels-last memory-reinterpret -> pointwise 1x1 conv -> ReLU -> sync
BatchNorm), data-parallel over batch across 8 NeuronCores.

Self-contained: hardcodes shapes from the problem spec; imports only the
system-installed `concourse` (Bass/Tile) stack.

Key layout trick vs the naive scheme: conv blocks are STRIDED, block l holds
spatial rows n = 28q + l (q = 0..111 partitions, l = 0..27 blocks). The 3x3
stencil still reduces to 3 banded matmuls per block (bands at q-offsets
{-2,0,+2} <-> dh, block delta <-> dw); h-borders drop out automatically via
band clipping, w-borders only touch the two wrap matrices (parity masks on q).
Because q is the SLOW index of the channels-last flat order
(F = 3584 q + 128 l + c), the conv output can be written to DRAM with 3584B
contiguous runs (full DMA speed) and read back as y = [128, 3136] contiguous,
realizing the reference's memory reinterpretation with ~1/3 the DMA cost of a
256B-chunked bounce.

Per core (8 of 64 batches, 2 groups of 4):
  1. Load x[b] [128c, 3136n] bf16. PE-transpose strided column sets
     x[:, l::28] -> xt_l [112q, 4b*128c] bf16 (DVE copies PSUM->SBUF).
  2. Conv: ps_l = A_{-1}^T xt_{l-1} + A_0^T xt_l + A_{+1}^T xt_{l+1} (wrap
     matrices for l=0/27), ACT copies f32 PSUM -> bf16 zg half-tiles.
  3. Bounce zg -> DRAM (4 half-writes/group, 3584B runs) and read back
     y[b] = [128, 3136] bf16 (contiguous).
  4. Pointwise out = pw @ y on PE (bf16 in, f32 accum); ReLU fused into
     PSUM->SBUF with per-channel sum accumulators; squares via second pass
     with sumsq accumulators. Pre-BN activations stay resident in SBUF bf16.
  5. Per-channel (sum, sumsq) AllGather across 8 cores + local reduce (exact
     sync-BN), affine applied on DVE/ACT/Pool, bf16 written out (host widens
     to f32).
"""

import os
import numpy as np
from contextlib import ExitStack

import concourse.bass as bass
import concourse.bacc as bacc
import concourse.mybir as mybir
from concourse import tile
from concourse.bass_utils import run_bass_kernel_spmd

F32 = mybir.dt.float32
BF16 = mybir.dt.bfloat16

B, CIN, COUT, H, W = 64, 128, 256, 56, 56
HW = H * W              # 3136
Q = 112                 # rows per strided block (n = 28q + l)
NBLK = 28               # blocks per image
HBLK = NBLK // 2        # 14 blocks per z half-tile
NCORES = 8
BPC = B // NCORES       # 8 batches per core
GB = 4                  # batches per conv group
NGRP = BPC // GB        # 2
NCH = 448               # pointwise moving chunk
EPS = 1e-5
NTOT = float(B * HW)    # BN count
NSL = 4                 # stats slots per (batch, oc)


def _build_A(nc, k_sb, const, tmp_pool):
    """Build the 5 banded conv matrices [128part(q_in, 112 used), 112(q_out)]
    bf16: A[dw] plain (bands q_in-q_out = 2*dh, coeff k[3*(dh+1)+(dw+1)]),
    plus wrap variants A[-1]w (shifts 2dh-1, odd q_out only; used by block 0
    reading xt_27) and A[+1]w (shifts 2dh+1, even q_out only; block 27
    reading xt_0)."""
    specs = {}
    # block-0's matrices first so conv can start as soon as possible
    specs["Am1w"] = [(2 * dh - 1, 3 * (dh + 1) + 0, 1) for dh in (-1, 0, 1)]
    for dw in (0, 1, -1):
        specs[f"A{dw}"] = [(2 * dh, 3 * (dh + 1) + (dw + 1), None)
                           for dh in (-1, 0, 1)]
    specs["Ap1w"] = [(2 * dh + 1, 3 * (dh + 1) + 2, 0) for dh in (-1, 0, 1)]

    out = {}
    for nm, taps in specs.items():
        parts = []
        for shift, k9i, parity in taps:
            t = tmp_pool.tile([128, Q], F32, tag="abuild", bufs=16,
                              name=f"ab_{nm}_{shift}")
            # diagonal: keep k where p - f - shift == 0 (f = 2*i1 + i2)
            nc.gpsimd.affine_select(
                out=t[:], in_=k_sb[:, k9i:k9i + 1].broadcast_to((128, Q)),
                pattern=[[-2, 56], [-1, 2]], base=-shift,
                compare_op=mybir.AluOpType.is_equal, fill=0.0,
                channel_multiplier=1)
            if parity is not None:
                # keep only columns with f % 2 == parity (value = i2 = f % 2)
                nc.gpsimd.affine_select(
                    out=t[:], in_=t[:], pattern=[[0, 56], [1, 2]],
                    base=-parity, compare_op=mybir.AluOpType.is_equal,
                    fill=0.0, channel_multiplier=0)
            parts.append(t)
        af = tmp_pool.tile([128, Q], F32, tag="abuildacc", bufs=2,
                           name=f"af_{nm}")
        nc.vector.tensor_tensor(af[:], parts[0][:], parts[1][:],
                                mybir.AluOpType.add)
        nc.vector.tensor_tensor(af[:], af[:], parts[2][:],
                                mybir.AluOpType.add)
        ar = const.tile([128, Q], BF16, tag=f"Ar_{nm}", name=f"Ar_{nm}")
        nc.vector.tensor_copy(ar[:], af[:])
        out[nm] = ar
    return out


def build_nc():
    nc = bacc.Bacc(num_devices=NCORES)

    x_in = nc.declare_dram_parameter("x", [BPC, CIN, HW], BF16, isOutput=False)
    dwk = nc.declare_dram_parameter("dwk", [9], F32, isOutput=False)
    pwt = nc.declare_dram_parameter("pwT", [CIN, COUT], F32, isOutput=False)
    gam = nc.declare_dram_parameter("gamma", [COUT], F32, isOutput=False)
    bet = nc.declare_dram_parameter("beta", [COUT], F32, isOutput=False)
    out = nc.declare_dram_parameter("out", [BPC, COUT, HW], BF16, isOutput=True)

    with ExitStack() as ctx:
        tc = ctx.enter_context(tile.TileContext(nc))
        const = ctx.enter_context(tc.tile_pool(name="const", bufs=1))
        xtpool = ctx.enter_context(tc.tile_pool(name="xt", bufs=4))
        zgpool = ctx.enter_context(tc.tile_pool(name="zg", bufs=2))
        ypool = ctx.enter_context(tc.tile_pool(name="y", bufs=3))
        respool = ctx.enter_context(tc.tile_pool(name="res", bufs=2 * BPC))
        scpool = ctx.enter_context(tc.tile_pool(name="scr", bufs=1))
        ps_t = ctx.enter_context(tc.tile_pool(name="ps_t", bufs=2, space="PSUM"))
        ps_m = ctx.enter_context(tc.tile_pool(name="ps_m", bufs=3, space="PSUM"))
        dram = ctx.enter_context(tc.tile_pool(name="dram", bufs=1, space="DRAM"))

        no_cc = bool(os.environ.get("BASS_NO_CC"))

        # ---- constants ----
        k_sb = const.tile([128, 9], F32, tag="k")
        nc.sync.dma_start(k_sb[:], dwk.ap().partition_broadcast(128))

        ident = const.tile([128, 128], BF16, tag="ident")
        pw_sb = const.tile([128, COUT], BF16, tag="pw")
        gb_sb = const.tile([128, 4], F32, tag="gb")
        nc.sync.dma_start(gb_sb[:, 0:2], gam.ap().rearrange("(j p) -> p j", p=128))
        nc.sync.dma_start(gb_sb[:, 2:4], bet.ap().rearrange("(j p) -> p j", p=128))

        ones = const.tile([128, 1], F32, tag="ones")
        nc.gpsimd.memset(ones[:], 1.0)
        with tc.tile_pool(name="abuild", bufs=2) as tmp_pool:
            pw_f32 = tmp_pool.tile([128, COUT], F32, tag="pwf", bufs=1)
            nc.sync.dma_start(pw_f32[:], pwt[:, :])
            nc.vector.tensor_copy(pw_sb[:], pw_f32[:])
            identf = tmp_pool.tile([128, 128], F32, tag="identf", bufs=1)
            nc.gpsimd.affine_select(
                out=identf[:], in_=ones[:].broadcast_to((128, 128)),
                pattern=[[-1, 128]], base=0,
                compare_op=mybir.AluOpType.is_equal, fill=0.0,
                channel_multiplier=1)
            nc.gpsimd.tensor_copy(ident[:], identf[:])
            A = _build_A(nc, k_sb, const, tmp_pool)

        xpool = ctx.enter_context(tc.tile_pool(name="x", bufs=7))

        # stats slots: sums one column per (batch, chunk-group); sumsq one
        # column per (batch, oc) unit (single full-width Square per unit)
        sums = [const.tile([128, BPC * NSL], F32, tag=f"sum{oc}",
                           name=f"sums{oc}") for oc in range(2)]
        sqs = [const.tile([128, BPC], F32, tag=f"sq{oc}",
                          name=f"sqs{oc}") for oc in range(2)]

        # DRAM bounce scratch: zscr[g][b4] flat == y[b] flat (n-major, 128c)
        zscr = [dram.tile([GB, Q, NBLK * 128], BF16, tag=f"zg{g}",
                          name=f"zscr{g}") for g in range(NGRP)]
        # two-phase stats all-gather: one per conv group so the first hides
        # under the second group's pointwise work
        st_in = [dram.tile([128, 4], F32, tag=f"stin{g}", name=f"st_in{g}")
                 for g in range(NGRP)]
        st_gather = [dram.tile([NCORES, 128, 4], F32, tag=f"stg{g}",
                               name=f"st_g{g}") for g in range(NGRP)]

        res_tiles = [[None] * 2 for _ in range(BPC)]
        x_tiles = [None] * BPC
        y_tiles = [None] * BPC

        def load_x(b):
            xt_ = xpool.tile([128, HW], BF16, tag="x", name=f"xh{b}")
            nc.sync.dma_start(xt_[:], x_in[b, :, :])
            x_tiles[b] = xt_

        for b in range(GB + 1):
            load_x(b)

        # ---------- conv group emission (generator: yields per block) ----------
        def emit_conv(g):
            xt_tiles = {}
            zgh = [None, None]
            b0 = GB * g

            def transpose_block(l):
                tps = ps_t.tile([Q, GB * 128], BF16, tag="tp")
                for b4 in range(GB):
                    xv = (x_tiles[b0 + b4][:, :]
                          .rearrange("c (q l) -> l c q", q=Q, l=NBLK)[l])
                    nc.tensor.transpose(
                        tps[:, 128 * b4:128 * (b4 + 1)], xv, ident[:])
                xt_sb = xtpool.tile([Q, GB * 128], BF16,
                                    tag=("xt27" if l == 27 else
                                         "xt0" if l == 0 else "xt"),
                                    bufs=(1 if l in (0, 27) else 4),
                                    name=f"xt{g}_{l}")
                nc.vector.tensor_copy(xt_sb[:], tps[:])
                xt_tiles[l] = xt_sb

            def conv_block(l):
                h = l // HBLK
                if zgh[h] is None:
                    zgh[h] = zgpool.tile([Q, GB * HBLK * 128], BF16, tag="zg",
                                         name=f"zgt{g}_{h}")
                psm = ps_m.tile([128, 1024], F32, tag="m", name="psc")
                ps = psm[0:Q, 0:GB * 128]
                if l == 0:
                    mats = [(A["Am1w"], 27), (A["A0"], 0), (A["A1"], 1)]
                elif l == NBLK - 1:
                    mats = [(A["A-1"], l - 1), (A["A0"], l), (A["Ap1w"], 0)]
                else:
                    mats = [(A["A-1"], l - 1), (A["A0"], l), (A["A1"], l + 1)]
                for idx, (a, src) in enumerate(mats):
                    nc.tensor.matmul(
                        ps[:], a[0:Q, :], xt_tiles[src][:],
                        start=(idx == 0), stop=(idx == 2))
                zv = (zgh[h][:, :]
                      .rearrange("q (b l c) -> l q b c", b=GB, l=HBLK, c=128)
                      [l % HBLK])
                nc.scalar.activation(zv, ps[:],
                                     mybir.ActivationFunctionType.Copy)
                if l % HBLK == HBLK - 1:
                    for bp in range(2):
                        nc.sync.dma_start(
                            zscr[g][2 * bp:2 * bp + 2,
                                    :, 1792 * h:1792 * (h + 1)]
                            .rearrange("b q s -> q b s"),
                            zgh[h][:, :].rearrange("q (b lc) -> q b lc", b=GB)
                            [:, 2 * bp:2 * bp + 2, :])
                    zgh[h] = None

            transpose_block(27)
            yield
            transpose_block(0)
            yield
            for l in range(1, NBLK):
                transpose_block(l)
                conv_block(l - 1)
                yield
            conv_block(NBLK - 1)
            yield

        # ---------- pointwise emission (per (b, oc) unit) ----------
        def emit_pointwise(g):
            for b4 in range(GB):
                b = GB * g + b4
                y_sb = ypool.tile([128, HW], BF16, tag="y", name=f"y{b}")
                nc.sync.dma_start(
                    y_sb[:],
                    zscr[g][b4].flatten().rearrange("(p n) -> p n", p=128))
                y_tiles[b] = y_sb
                for oc in range(2):
                    res = respool.tile([128, HW], BF16, tag="res")
                    res_tiles[b][oc] = res
                    for jj, js in enumerate([(0, 1), (2, 3), (4, 5), (6,)]):
                        w = NCH * len(js)
                        ps = ps_m.tile([128, 1024], F32, tag="m", name="psp")
                        for k, j in enumerate(js):
                            nc.tensor.matmul(
                                ps[:, 512 * k:512 * k + NCH],
                                pw_sb[:, 128 * oc:128 * (oc + 1)],
                                y_sb[:, NCH * j:NCH * (j + 1)],
                                start=True, stop=True)
                        slot = b * NSL + jj
                        j0 = js[0]
                        if len(js) == 2:
                            ps_in = (ps[:].rearrange("p (k c) -> p k c", c=512)
                                     [:, :, 0:NCH])
                            rs = (res[:, NCH * j0:NCH * j0 + w]
                                  .rearrange("p (k c) -> p k c", c=NCH))
                        else:
                            ps_in = ps[:, 0:NCH]
                            rs = res[:, NCH * j0:NCH * j0 + w]
                        nc.vector.tensor_scalar(
                            rs, ps_in, 0.0, 0.0,
                            mybir.AluOpType.max, mybir.AluOpType.add,
                            accum_out=sums[oc][:, slot:slot + 1])
                    # one full-width square pass on ACT -> sumsq
                    sc = scpool.tile([128, HW], BF16, tag="scr")
                    nc.scalar.activation(
                        sc[:], res[:],
                        mybir.ActivationFunctionType.Square,
                        accum_out=sqs[oc][:, b:b + 1])
                    yield

        # per-group stats launch: reduce this group's slots and AllGather
        red = [const.tile([128, 4], F32, tag=f"red{g}", name=f"red{g}")
               for g in range(NGRP)]
        ag = [const.tile([128, 4 * NCORES], F32, tag=f"ag{g}", name=f"agt{g}")
              for g in range(NGRP)]

        def emit_stats_gather(g):
            s0, s1 = GB * NSL * g, GB * NSL * (g + 1)
            for oc in range(2):
                nc.vector.tensor_reduce(red[g][:, oc:oc + 1],
                                        sums[oc][:, s0:s1],
                                        axis=mybir.AxisListType.X,
                                        op=mybir.AluOpType.add)
                nc.vector.tensor_reduce(red[g][:, 2 + oc:3 + oc],
                                        sqs[oc][:, GB * g:GB * (g + 1)],
                                        axis=mybir.AxisListType.X,
                                        op=mybir.AluOpType.add)
            nc.gpsimd.dma_start(st_in[g][:], red[g][:])
            if no_cc:
                for r in range(NCORES):
                    nc.gpsimd.dma_start(st_gather[g][r], st_in[g][:])
            else:
                nc.gpsimd.collective_compute(
                    "AllGather", mybir.AluOpType.bypass,
                    replica_groups=[list(range(NCORES))],
                    ins=[st_in[g][:].opt()], outs=[st_gather[g][:].opt()])
            nc.gpsimd.dma_start(
                ag[g][:], st_gather[g][:].rearrange("r p f -> p r f"))

        # ---------- schedule: g0 conv | g1 conv x g0 pw interleave | g1 pw ----
        for _ in emit_conv(0):
            pass
        for b in range(GB + 1, BPC):
            load_x(b)
        pw0 = emit_pointwise(0)
        step = 0
        for _ in emit_conv(1):
            step += 1
            if step % 5 == 0:
                next(pw0, None)
        # remaining pw0 units drain into the g1-bounce seam
        for _ in pw0:
            pass
        emit_stats_gather(0)
        for _ in emit_pointwise(1):
            pass
        emit_stats_gather(1)

        # ---- combine gathered stats -> affine params ----
        me = const.tile([128, 4], F32, tag="me")    # mean0 mean1 msq0 msq1
        var = const.tile([128, 2], F32, tag="var")
        std = const.tile([128, 2], F32, tag="std")
        rstd = const.tile([128, 2], F32, tag="rstd")
        sc_b = const.tile([128, 4], F32, tag="scb")  # scale0 scale1 nbias0 nbias1

        a0, a1 = ag[0], ag[1]
        nc.vector.tensor_tensor(a0[:, 0:32], a0[:, 0:32], a1[:, 0:32],
                                mybir.AluOpType.add)
        nc.vector.tensor_tensor(a0[:, 0:16], a0[:, 0:16], a0[:, 16:32],
                                mybir.AluOpType.add)
        nc.vector.tensor_tensor(a0[:, 0:8], a0[:, 0:8], a0[:, 8:16],
                                mybir.AluOpType.add)
        nc.vector.tensor_tensor(a0[:, 0:4], a0[:, 0:4], a0[:, 4:8],
                                mybir.AluOpType.add)

        nc.vector.tensor_scalar(me[:], a0[:, 0:4], 1.0 / NTOT, None,
                                mybir.AluOpType.mult)
        nc.vector.tensor_tensor(var[:], me[:, 0:2], me[:, 0:2],
                                mybir.AluOpType.mult)
        nc.vector.tensor_tensor(var[:], me[:, 2:4], var[:],
                                mybir.AluOpType.subtract)
        nc.vector.tensor_scalar(var[:], var[:], EPS, None,
                                mybir.AluOpType.add)
        nc.scalar.activation(std[:], var[:],
                             mybir.ActivationFunctionType.Sqrt)
        nc.vector.reciprocal(rstd[:], std[:])
        nc.vector.tensor_tensor(sc_b[:, 0:2], rstd[:], gb_sb[:, 0:2],
                                mybir.AluOpType.mult)
        nc.vector.tensor_tensor(sc_b[:, 2:4], me[:, 0:2], sc_b[:, 0:2],
                                mybir.AluOpType.mult)
        nc.vector.tensor_tensor(sc_b[:, 2:4], gb_sb[:, 2:4], sc_b[:, 2:4],
                                mybir.AluOpType.subtract)

        # ---- phase 2: in-place affine on res + bf16 writeout ----
        for b in range(BPC):
            for oc in range(2):
                r = res_tiles[b][oc]
                idx = 2 * b + oc
                if idx % 8 == 3:
                    nc.gpsimd.tensor_scalar(
                        r[:], r[:],
                        sc_b[:, oc:oc + 1], sc_b[:, 2 + oc:3 + oc],
                        mybir.AluOpType.mult, mybir.AluOpType.add)
                elif idx % 4 == 1:
                    nc.scalar.activation(
                        r[:], r[:],
                        mybir.ActivationFunctionType.Identity,
                        bias=sc_b[:, 2 + oc:3 + oc],
                        scale=sc_b[:, oc:oc + 1])
                else:
                    nc.vector.tensor_scalar(
                        r[:], r[:],
                        sc_b[:, oc:oc + 1], sc_b[:, 2 + oc:3 + oc],
                        mybir.AluOpType.mult, mybir.AluOpType.add)
                nc.sync.dma_start(out[b, 128 * oc:128 * (oc + 1), :], r[:])

    nc.finalize()
    return nc


_NC_CACHE = []


def kernel(x, dw_w, pw_w, gamma, beta):
    import ml_dtypes
    x = np.ascontiguousarray(
        np.asarray(x, dtype=np.float32).astype(ml_dtypes.bfloat16)
    ).reshape(B, CIN, HW)
    dwk = np.ascontiguousarray(np.asarray(dw_w, dtype=np.float32)).reshape(9)
    pwT = np.ascontiguousarray(np.asarray(pw_w, dtype=np.float32).T)
    gamma = np.ascontiguousarray(np.asarray(gamma, dtype=np.float32))
    beta = np.ascontiguousarray(np.asarray(beta, dtype=np.float32))

    if not _NC_CACHE:
        _NC_CACHE.append(build_nc())
    nc = _NC_CACHE[0]

    in_maps = []
    for r in range(NCORES):
        shard = np.ascontiguousarray(x[r * BPC:(r + 1) * BPC])
        in_maps.append({"x": shard, "dwk": dwk, "pwT": pwT,
                        "gamma": gamma, "beta": beta})

    br = run_bass_kernel_spmd(nc, in_maps, list(range(NCORES)))
    outs = [np.asarray(br.results[r]["out"], dtype=np.float32)
            .reshape(BPC, COUT, H, W) for r in range(NCORES)]
    return np.concatenate(outs, axis=0)
